# revision 35
# baseline (speedup 1.0000x reference)
"""MoDA attention Trainium2 kernel, 8-way head-parallel. v2

Sharding: core c owns Q heads {2c, 2c+1} and K/V head c (their GQA group),
plus that K head's depth caches. Each core computes its heads' projections,
RoPE, joint seq+depth softmax attention, and a partial output projection
(rows 2c*128:(2c+2)*128 of Wo). Host sums the 8 partial outputs.

Device layouts (host-prepared, bf16, all DMA-contiguous):
  xT   [DM, T]        x transposed
  wq   [128, DK, 2*HD] Wq columns for this core's 2 Q heads (kt-blocked)
  wk   [128, DK, HD], wv [128, DK, HD]
  wo   [128, NQH, DM] Wo rows for the 2 heads
  cosT/sinT [HD, T]   rope tables transposed
  kdT/vdT [L, HD, T]  depth caches transposed (per K head)
  mask [128, 128]     upper-tri keep-mask for causal diagonal blocks

v2 changes vs v1 (213us):
 - Z row-sums: per-jb ones^T@U matmuls (25us PE) replaced by U-tile presum
   on DVE (fp32 acc) + GpSimd (bf16 acc) and ONE fp32r ones-matmul per unit.
 - depth-z: 4x 1*1*512 K=1 matmuls -> dl stacked into [4,TC] psum rows, one
   [4,TC] exp (1 scalar op vs 4 row-exps), one K=4 ones-matmul.
 - PE order per unit: S stream, dl, O stream, then Z (so presum latency
   never stalls the PE).
 - epilogue split DVE/GpSimd; outproj PSUM->SBUF copies split scalar/DVE.
 - startup: first-needed DMAs (wk, xT row 0) issued first on separate
   queues (sync/scalar/vector).
 - tail: last chunk's epilogue final-mul + outproj + stores done per
   128-row block, stores spread over 3 DMA queues.
"""

import os
import sys

sys.path.insert(0, "/opt/trn_rl_repo")

import numpy as np
import ml_dtypes

import concourse.bass as bass
import concourse.tile as tile
import concourse.mybir as mybir
from concourse import bacc
from concourse.bass_utils import run_bass_kernel_spmd

BF16 = mybir.dt.bfloat16
FP32 = mybir.dt.float32
FP32R = mybir.dt.float32r
NPBF16 = ml_dtypes.bfloat16

HQ, HK, HD, DM = 16, 8, 128, 2048
L = 4
GQA = HQ // HK
SCALE = float(HD) ** -0.5
N_CORES = 8
NQH = 2  # Q heads per core
TC = 512  # T chunk (free dim of most matmuls)
DK = DM // 128  # contraction tiles

# presum split: gpsimd sums the first GP_SHARE[jmax] U tiles
GP_SHARE = {4: 0, 8: 3, 12: 4, 16: 6}

_programs = {}
last_result = None


def _ts(i, n):
    return bass.ts(i, n)


def build_program(T):
    nc = bacc.Bacc(
        "TRN2",
        target_bir_lowering=False,
        debug=False,
        enable_asserts=False,
        num_devices=N_CORES,
    )

    xT = nc.dram_tensor("xT", [DM, T], BF16, kind="ExternalInput").ap()
    wq = nc.dram_tensor("wq", [128, DK, NQH * HD], BF16, kind="ExternalInput").ap()
    wk = nc.dram_tensor("wk", [128, DK, HD], BF16, kind="ExternalInput").ap()
    wv = nc.dram_tensor("wv", [128, DK, HD], BF16, kind="ExternalInput").ap()
    wo = nc.dram_tensor("wo", [128, NQH, DM], BF16, kind="ExternalInput").ap()
    cosT = nc.dram_tensor("cosT", [HD, T], BF16, kind="ExternalInput").ap()
    sinT = nc.dram_tensor("sinT", [HD, T], BF16, kind="ExternalInput").ap()
    kdT = nc.dram_tensor("kdT", [L, HD, T], BF16, kind="ExternalInput").ap()
    vdT = nc.dram_tensor("vdT", [L, HD, T], BF16, kind="ExternalInput").ap()
    mask = nc.dram_tensor("mask", [128, 128], BF16, kind="ExternalInput").ap()
    ident = nc.dram_tensor("ident", [128, 128], BF16, kind="ExternalInput").ap()
    out = nc.dram_tensor("out", [T, DM], BF16, kind="ExternalOutput").ap()

    NC_CHUNKS = T // TC  # i-chunks per head
    NTB = T // 128  # 128-blocks in T
    TPC = TC // 128  # 128-blocks per chunk

    with tile.TileContext(nc) as tc:
        with tc.tile_pool(name="const", bufs=1) as cp:
            # ---- persistent SBUF tensors ----
            wq_sb = cp.tile([128, DK, NQH * HD], BF16)
            wk_sb = cp.tile([128, DK, HD], BF16)
            wv_sb = cp.tile([128, DK, HD], BF16)
            wo_sb = cp.tile([128, NQH, DM], BF16)
            cos_sb = cp.tile([128, T], BF16)
            sin_sb = cp.tile([128, T], BF16)
            kdT_sb = cp.tile([128, L, T], BF16)
            vdT_sb = cp.tile([128, L, T], BF16)
            mask_sb = cp.tile([128, 128], BF16)
            ident_sb = cp.tile([128, 128], BF16)
            ones_sb = cp.tile([128, 1], BF16)
            nc.vector.memset(ones_sb[:], 1.0)
            onesf_sb = cp.tile([128, 1], FP32)
            nc.vector.memset(onesf_sb[:], 1.0)
            ones1_sb = cp.tile([1, 128], BF16)
            nc.vector.memset(ones1_sb[:], 1.0)

            qT_sb = cp.tile([128, NQH, T], BF16)  # rope'd Q^T per head
            kT_sb = cp.tile([128, T], BF16)  # rope'd K^T
            v_sb = cp.tile([128, NTB, HD], BF16)  # V natural, [i-in-block, block, d]
            oT_sb = cp.tile([128, NQH, T], BF16)  # normalized O^T per head

            # ---- phase A: projections + rope ----
            with tc.tile_pool(name="psA", bufs=7, space="PSUM") as psA, \
                 tc.tile_pool(name="psAv", bufs=1, space="PSUM") as psAv, \
                 tc.tile_pool(name="cpX", bufs=1) as cpX, \
                 tc.tile_pool(name="sA", bufs=3) as sA:
                xT_sb = cpX.tile([128, DK, T], BF16)
                # first-needed loads first, in small pieces, on 3 queues
                nc.sync.dma_start(wk_sb[:, 0:4, :], wk[:, 0:4, :])
                for cc in range(NC_CHUNKS):
                    eng = (nc.gpsimd, nc.scalar)[cc % 2]
                    eng.dma_start(xT_sb[:, 0, _ts(cc, TC)],
                                  xT[_ts(0, 128), _ts(cc, TC)])
                nc.sync.dma_start(wk_sb[:, 4:DK, :], wk[:, 4:DK, :])
                for kt in range(1, DK):
                    eng = (nc.sync, nc.gpsimd, nc.scalar)[kt % 3]
                    eng.dma_start(xT_sb[:, kt, :], xT[_ts(kt, 128), :])
                nc.gpsimd.dma_start(wq_sb[:], wq[:])
                nc.sync.dma_start(wv_sb[:], wv[:])
                nc.gpsimd.dma_start(cos_sb[:], cosT[:])
                nc.sync.dma_start(sin_sb[:], sinT[:])
                for l in range(L):
                    nc.sync.dma_start(kdT_sb[:, l, :], kdT[l])
                for l in range(L):
                    nc.gpsimd.dma_start(vdT_sb[:, l, :], vdT[l])
                nc.gpsimd.dma_start(mask_sb[:], mask[:])
                nc.sync.dma_start(ident_sb[:], ident[:])
                nc.sync.dma_start(wo_sb[:], wo[:])

                def rope_chunk(ps, dst, c):
                    cs = cos_sb[:, _ts(c, TC)]
                    sn = sin_sb[:, _ts(c, TC)]
                    praw = sA.tile([128, TC], BF16, tag="praw")
                    nc.scalar.copy(praw[:], ps[:])
                    prot = sA.tile([128, TC], BF16, tag="prot")
                    nc.vector.tensor_copy(prot[0:64, :], praw[64:128, :])
                    nc.vector.tensor_copy(prot[64:128, :], praw[0:64, :])
                    t1 = sA.tile([128, TC], BF16, tag="t1")
                    nc.vector.tensor_mul(t1[:], praw[:], cs)
                    t2 = sA.tile([128, TC], BF16, tag="t2")
                    nc.vector.tensor_mul(t2[:], prot[:], sn)
                    nc.vector.tensor_tensor(
                        dst[0:64, :], t1[0:64, :], t2[0:64, :],
                        op=mybir.AluOpType.subtract,
                    )
                    nc.vector.tensor_add(dst[64:128, :], t1[64:128, :], t2[64:128, :])

                # K and 3 of Q-head0's chunks jointly kt-outer: 7 chunk
                # psums of matmul work hide under the DMA-bound xT stream
                NJQ = 3
                pss7 = [psA.tile([128, TC], FP32, tag="proj", name=f"kq{i}")
                        for i in range(NC_CHUNKS + NJQ)]
                for kt in range(DK):
                    for c in range(NC_CHUNKS):
                        nc.tensor.matmul(
                            pss7[c][:], wk_sb[:, kt, :],
                            xT_sb[:, kt, _ts(c, TC)],
                            start=(kt == 0), stop=(kt == DK - 1),
                        )
                        if c < NJQ:
                            nc.tensor.matmul(
                                pss7[NC_CHUNKS + c][:],
                                wq_sb[:, kt, _ts(0, HD)],
                                xT_sb[:, kt, _ts(c, TC)],
                                start=(kt == 0), stop=(kt == DK - 1),
                            )
                for c in range(NC_CHUNKS):
                    rope_chunk(pss7[c], kT_sb[:, _ts(c, TC)], c)
                for c in range(NJQ):
                    rope_chunk(pss7[NC_CHUNKS + c], qT_sb[:, 0, _ts(c, TC)], c)

                def v_block(tb):
                    # one tb of V proj; LDWEIGHTS-bound, interleaved into Q
                    vp = psAv.tile([128, HD], FP32, tag="vp")
                    for kt in range(DK):
                        nc.tensor.matmul(
                            vp[:], xT_sb[:, kt, _ts(tb, 128)], wv_sb[:, kt, :],
                            start=(kt == 0), stop=(kt == DK - 1),
                        )
                    nc.scalar.copy(v_sb[:, tb, :], vp[:])

                # Q head 1 (+ Q-head0's last chunk) with V tb-blocks
                # interleaved (V's LDWEIGHTS hide under the 512-col matmuls)
                vq = 0
                pss = [psA.tile([128, TC], FP32, tag="proj", name=f"q{i}")
                       for i in range(NC_CHUNKS + 1)]
                for kt in range(DK):
                    for c in range(NC_CHUNKS):
                        nc.tensor.matmul(
                            pss[c][:], wq_sb[:, kt, _ts(1, HD)],
                            xT_sb[:, kt, _ts(c, TC)],
                            start=(kt == 0), stop=(kt == DK - 1),
                        )
                    nc.tensor.matmul(
                        pss[NC_CHUNKS][:], wq_sb[:, kt, _ts(0, HD)],
                        xT_sb[:, kt, _ts(NJQ, TC)],
                        start=(kt == 0), stop=(kt == DK - 1),
                    )
                    if vq < NTB:
                        v_block(vq)
                        vq += 1
                rope_chunk(pss[NC_CHUNKS], qT_sb[:, 0, _ts(NJQ, TC)], NJQ)
                for c in range(NC_CHUNKS):
                    rope_chunk(pss[c], qT_sb[:, 1, _ts(c, TC)], c)
                while vq < NTB:
                    v_block(vq)
                    vq += 1

            # ---- phase B: attention, software-pipelined emission ----
            with tc.tile_pool(name="psS", bufs=3, space="PSUM") as psS, \
                 tc.tile_pool(name="psO", bufs=3, space="PSUM") as psO, \
                 tc.tile_pool(name="psZ", bufs=1, space="PSUM") as psZ, \
                 tc.tile_pool(name="sBc", bufs=9) as sBc, \
                 tc.tile_pool(name="sZb", bufs=2) as sZb, \
                 tc.tile_pool(name="sB", bufs=22) as sB, \
                 tc.tile_pool(name="sEu", bufs=6) as sEu, \
                 tc.tile_pool(name="sAcc", bufs=3) as sAcc, \
                 tc.tile_pool(name="sWd", bufs=9) as sWd, \
                 tc.tile_pool(name="sB2", bufs=2) as sB2, \
                 tc.tile_pool(name="sC", bufs=6) as sC:

                def attention_part(c, h):
                    qch = qT_sb[:, h, _ts(c, TC)]
                    o_ps = psO.tile([128, TC], FP32, tag="o")
                    # two alternating 1-row psum tiles for dl; z reuses zp[0]
                    zp = [psZ.tile([1, TC], FP32, tag="z1", name="z1"),
                          psZ.tile([1, TC], FP32, tag="z2", name="z2")]
                    jmax = (c + 1) * TPC
                    c0 = c * TPC
                    offs = [max(0, (jb - c0)) * 128 for jb in range(jmax)]
                    # dl_l matmul emitted after S-matmul dl_pos[l] (spreads
                    # the exp-gated WAR on the two shared psum rows)
                    step = max(1, (jmax - 2) // 4)
                    dl_pos = [min(jmax - 1, 1 + l * step) for l in range(L)]
                    wd = []
                    bcs = []
                    # eu tiles upfront: l 0,1 on gpsimd, l 2,3 on DVE
                    eus = []
                    for l in range(L):
                        eu = sEu.tile([128, TC], BF16, tag="eu", name=f"eu{l}")
                        nc.vector.tensor_mul(eu[:], qch, kdT_sb[:, l, _ts(c, TC)])
                        eus.append(eu)

                    def emit_dl(l):
                        nc.tensor.matmul(
                            zp[l % 2][:], ones_sb[:], eus[l][:],
                            start=True, stop=True,
                        )
                        wdl = sWd.tile([1, TC], BF16, tag="wd", name=f"wd{l}")
                        nc.scalar.activation(
                            wdl[:], zp[l % 2][:],
                            mybir.ActivationFunctionType.Exp, scale=SCALE,
                        )
                        wd.append(wdl)
                        bc = sBc.tile([128, TC], BF16, tag="bc", name=f"bc{l}")
                        nc.gpsimd.partition_broadcast(bc[:], wdl[0:1, :])
                        bcs.append(bc)

                    # --- PE: S stream (dl interleaved, TRI-seeded diag) ---
                    # presum chains: gpsimd sums the first ng tiles (ready
                    # earliest), DVE the rest; merge on DVE
                    acc_v = sAcc.tile([128, TC], BF16, tag="av")
                    us = []
                    for jb in range(jmax):
                        off = offs[jb]
                        s_ps = psS.tile([128, TC], FP32, tag="s")
                        if jb >= c0:
                            # additive causal mask seed on the diagonal block
                            nc.tensor.matmul(
                                s_ps[:, off : off + 128], ident_sb[:],
                                mask_sb[:], start=True, stop=False,
                            )
                            nc.tensor.matmul(
                                s_ps[:, off : off + 128],
                                kT_sb[:, _ts(jb, 128)],
                                qch[:, off : off + 128],
                                start=False, stop=True,
                            )
                            if off + 128 < TC:
                                nc.tensor.matmul(
                                    s_ps[:, off + 128 : TC],
                                    kT_sb[:, _ts(jb, 128)],
                                    qch[:, off + 128 : TC],
                                    start=True, stop=True,
                                )
                        else:
                            nc.tensor.matmul(
                                s_ps[:, 0:TC], kT_sb[:, _ts(jb, 128)],
                                qch[:, 0:TC], start=True, stop=True,
                            )
                        u = sB.tile([128, TC], BF16, tag="u")
                        nc.scalar.activation(
                            u[:, off:TC], s_ps[:, off:TC],
                            mybir.ActivationFunctionType.Exp, scale=SCALE,
                        )
                        us.append(u)
                        # presum: only each tile's valid region [off:TC];
                        # acc regions left of off stay as previously summed
                        if jb == 0:
                            if jmax == TPC:  # c == 0: lone full tile, copy
                                nc.vector.tensor_copy(acc_v[:], us[0][:])
                        elif jb == 1:
                            if jmax == TPC:
                                nc.vector.tensor_add(
                                    acc_v[:, off:TC], acc_v[:, off:TC],
                                    us[1][:, off:TC],
                                )
                            else:
                                nc.vector.tensor_add(
                                    acc_v[:], us[0][:], us[1][:]
                                )
                        else:
                            nc.vector.tensor_add(
                                acc_v[:, off:TC], acc_v[:, off:TC],
                                us[jb][:, off:TC],
                            )
                        for l in range(L):
                            if dl_pos[l] == jb:
                                emit_dl(l)
                        if jb >= 2:
                            oj = jb - 2
                            nc.tensor.matmul(
                                o_ps[:, offs[oj]:TC], v_sb[:, oj, :],
                                us[oj][:, offs[oj]:TC],
                                start=(oj == 0), stop=False,
                            )
                    u_sum = acc_v
                    for oj in range(max(0, jmax - 2), jmax):
                        nc.tensor.matmul(
                            o_ps[:, offs[oj]:TC], v_sb[:, oj, :],
                            us[oj][:, offs[oj]:TC],
                            start=(oj == 0), stop=(oj == jmax - 1),
                        )
                    return o_ps, bcs, zp, u_sum, wd, us, offs, c0

                def attention_finish(o_ps, bcs, zp, u_sum, wd, us, offs, c0):
                    nc.tensor.matmul(
                        zp[0][:], ones_sb[:], u_sum[:],
                        start=True, stop=False,
                    )
                    for l in range(L):
                        nc.tensor.matmul(
                            zp[0][:], ones1_sb[0:1, 0:1], wd[l][:],
                            start=False, stop=(l == L - 1),
                        )
                    zinv = sB2.tile([1, TC], FP32, tag="zi")
                    nc.vector.reciprocal_approx_fast(zinv[:], zp[0][:])
                    zb = sZb.tile([128, TC], FP32, tag="zb")
                    nc.gpsimd.partition_broadcast(zb[:], zinv[0:1, :])
                    return o_ps, bcs, zb

                def epilogue_part(c, h, o_ps, bcs, zb, split_tb=False):
                    vd = lambda l: vdT_sb[:, l, _ts(c, TC)]
                    acc_a = sB2.tile([128, TC], BF16, tag="ea")
                    tmp_a = sB2.tile([128, TC], BF16, tag="eta")
                    for l in range(L):
                        dst = acc_a if l == 0 else tmp_a
                        nc.vector.tensor_mul(dst[:], vd(l), bcs[l][:])
                        if l > 0:
                            nc.vector.tensor_add(acc_a[:], acc_a[:], tmp_a[:])
                    o_sum = sB2.tile([128, TC], FP32, tag="osum")
                    nc.vector.tensor_add(o_sum[:], o_ps[:], acc_a[:])
                    if not split_tb:
                        nc.vector.tensor_mul(
                            oT_sb[:, h, _ts(c, TC)], o_sum[:], zb[:]
                        )
                    else:
                        for tbl in range(TPC):
                            sl = slice(tbl * 128, (tbl + 1) * 128)
                            nc.vector.tensor_mul(
                                oT_sb[:, h, c * TC + tbl * 128 :
                                      c * TC + (tbl + 1) * 128],
                                o_sum[:, sl], zb[:, sl],
                            )

                def outproj_tb(tb, fine=False):
                    # out rows tb*128:(tb+1)*128, all DM cols
                    for nchunk in range(DM // TC):
                        op = psS.tile([128, TC], FP32, tag="s", name="op")
                        for h in range(NQH):
                            nc.tensor.matmul(
                                op[:], oT_sb[:, h, _ts(tb, 128)],
                                wo_sb[:, h, _ts(nchunk, TC)],
                                start=(h == 0), stop=(h == NQH - 1),
                            )
                        res = sC.tile([128, TC], BF16, tag="res")
                        if (tb + nchunk) % 2 == 0:
                            nc.scalar.copy(res[:], op[:])
                        else:
                            nc.vector.tensor_copy(res[:], op[:])
                        eng = (nc.sync, nc.scalar)[nchunk % 2] \
                            if fine else nc.sync
                        eng.dma_start(out[_ts(tb, 128), _ts(nchunk, TC)], res[:])

                def outproj_chunk(c, half=None, fine=False):
                    rng = range(TPC)
                    if half is not None:
                        rng = rng[:2] if half == 0 else rng[2:]
                    for tbl in rng:
                        outproj_tb(c * TPC + tbl, fine=fine)

                units = [(c, h) for c in reversed(range(NC_CHUNKS))
                         for h in range(NQH)]
                pend_epi = []  # (c, h, o_ps, bcs, zb)
                pend_out = []  # (chunk, next_half)
                for idx, (c, h) in enumerate(units):
                    sa = attention_part(c, h)
                    if pend_out:
                        pc2, half = pend_out[0]
                        outproj_chunk(pc2, half)
                        if half == 1:
                            pend_out.pop(0)
                        else:
                            pend_out[0] = (pc2, 1)
                    state = attention_finish(*sa)
                    if pend_epi:
                        pc, ph, po, pbcs, pzb = pend_epi.pop(0)
                        epilogue_part(pc, ph, po, pbcs, pzb)
                        if ph == NQH - 1:
                            pend_out.append((pc, 0))
                    pend_epi.append((c, h) + tuple(state))
                # drain: last chunk handled per-tb so the tail is short
                while pend_epi:
                    pc, ph, po, pbcs, pzb = pend_epi.pop(0)
                    epilogue_part(pc, ph, po, pbcs, pzb,
                                  split_tb=(pc == 0 and ph == NQH - 1))
                    if ph == NQH - 1:
                        pend_out.append((pc, 0))
                while pend_out:
                    pc2 = pend_out.pop(0)[0]
                    outproj_chunk(pc2, fine=(pc2 == 0))

    nc.compile()
    return nc


def get_program(T):
    if T not in _programs:
        _programs[T] = build_program(T)
    return _programs[T]


def make_in_maps(x, depth_k, depth_v, cos, sin, Wq, Wk, Wv, Wo, T):
    xT16 = np.ascontiguousarray(x[0].T).astype(NPBF16)
    cosT16 = np.ascontiguousarray(cos[0, 0].T).astype(NPBF16)
    sinT16 = np.ascontiguousarray(sin[0, 0].T).astype(NPBF16)
    mask16 = np.where(np.triu(np.ones((128, 128), np.float32)) > 0,
                      0.0, -30000.0).astype(np.float32).astype(NPBF16)
    ident16 = np.eye(128, dtype=np.float32).astype(NPBF16)

    def ktblock(w):  # [DM, m] -> [128, DK, m]
        m = w.shape[1]
        return np.ascontiguousarray(
            w.reshape(DK, 128, m).transpose(1, 0, 2)
        ).astype(NPBF16)

    in_maps = []
    for c in range(N_CORES):
        wq_c = ktblock(Wq[:, 2 * c * HD : (2 * c + 2) * HD])
        wk_c = ktblock(Wk[:, c * HD : (c + 1) * HD])
        wv_c = ktblock(Wv[:, c * HD : (c + 1) * HD])
        wo_c = np.ascontiguousarray(
            Wo[2 * c * HD : (2 * c + 2) * HD, :].reshape(NQH, HD, DM)
            .transpose(1, 0, 2)
        ).astype(NPBF16)
        kdT_c = np.ascontiguousarray(depth_k[:, 0, c].transpose(0, 2, 1)).astype(NPBF16)
        vdT_c = np.ascontiguousarray(depth_v[:, 0, c].transpose(0, 2, 1)).astype(NPBF16)
        in_maps.append(
            {
                "xT": xT16, "wq": wq_c, "wk": wk_c, "wv": wv_c, "wo": wo_c,
                "cosT": cosT16, "sinT": sinT16, "kdT": kdT_c, "vdT": vdT_c,
                "mask": mask16, "ident": ident16,
            }
        )
    return in_maps


def kernel(x, depth_k, depth_v, cos, sin, Wq, Wk, Wv, Wo):
    x = np.asarray(x, np.float32)
    T = x.shape[1]
    nc = get_program(T)
    in_maps = make_in_maps(
        x, np.asarray(depth_k, np.float32), np.asarray(depth_v, np.float32),
        np.asarray(cos, np.float32), np.asarray(sin, np.float32),
        np.asarray(Wq, np.float32), np.asarray(Wk, np.float32),
        np.asarray(Wv, np.float32), np.asarray(Wo, np.float32), T,
    )
    trace = bool(os.environ.get("MODA_TRACE"))
    res = run_bass_kernel_spmd(nc, in_maps, list(range(N_CORES)), trace=trace)
    global last_result
    last_result = res
    total = np.zeros((T, DM), np.float32)
    for c in range(N_CORES):
        total += res.results[c]["out"].astype(np.float32)
    return total.reshape(1, T, DM)
